# revision 30
# baseline (speedup 1.0000x reference)
"""Trainium2 Bass kernel for nn_LocalInteractionsLayer.

Reference computation:
    seq_pairs [B=16, C=8, L=4096, 2] f32
    top = seq_pairs[..., 0]; bot = seq_pairs[..., 1]
    out[b, p, c*225 + i*15 + j] = top[b, c, p+i] * bot[b, c, p+j]
    for p in [0, P), i,j in [0, 15), P = L - 14 = 4082
    -> out [16, 4082, 1800] f32 (~470 MB; heavily output-write bound).

Strategy (v3 — fp16, 3-engine compute split, on-device window expansion):
  - Data-parallel over batch: 2 batches per core on 8 cores.
  - fp16 end-to-end on device (output upcast to f32 on the host; ~4e-4 rel
    error against the 2e-2 gate) halves the dominant store traffic.
  - The 64 per-core broadcast-AP multiplies run 2:1 on DVE:GPSIMD (measured
    ~2.3 us vs ~4.6 us per [128, 1800] tile). DVE reads its top-window
    operand from PSUM so its tensor_mul stays off the DVE/GPSIMD *shared*
    SBUF port (an exclusive per-instruction lock) and both engines overlap.
  - Window tiles are expanded on device instead of loaded: the tensor engine
    multiplies banded shift matrices against the compact raw sequences
    (wrap-around handled by a second accumulating matmul), writing windows
    for DVE tiles into PSUM; the scalar engine reorders bottom windows
    PSUM->SBUF. GPSIMD tiles (no PSUM access) read host-packed window slices
    for just their tiles. Per-core loads drop 15.7 MB (f32 v1) -> 1.8 MB,
    leaving a nearly pure-write HBM stream (fewer read/write turnarounds).
  - Output tiles are fused in pairs: one ~905 KB store per 2 tiles,
    alternating between the SP and ACT HWDGE rings (the ACT ring is nearly
    free once window loads are gone); the small raw/window loads ride the
    ACT ring. A dormant _build_v4 (K_MODE=v4) adds an ACT-engine multiply
    path via the (x+y)^2 identity — numerically exact but a net loss on HW
    (in-order engine queues couple DVE to the PE<->ACT chain); kept for
    reference.
"""

import os
import sys

if "/opt/trn_rl_repo" not in sys.path:
    sys.path.insert(0, "/opt/trn_rl_repo")

import numpy as np
from numpy.lib.stride_tricks import sliding_window_view

import concourse.tile as tile
from concourse import bacc, mybir
from concourse.bass_utils import run_bass_kernel_spmd

W = 15            # window length (2*7+1)
WPAD = W - 1
B, C, L = 16, 8, 4096
P = L - WPAD      # 4082 valid output positions
FREE = C * W * W  # 1800
NCORES = 8
BPC = B // NCORES  # batches per core = 2
NT = L // 128      # 32 position-tiles per batch (last one partially valid)
NG = 4             # tile groups per batch
GT = NT // NG      # 8 tiles per group
GW = GT * C * W    # free size of one operand group = 960
CW = C * W         # 120 window values per position per operand
TCOL = NT + 1      # raw columns per channel incl. one zero-pad tile column
NGS = 3            # max GPSIMD tiles per group (pattern t % 3 == 2)
QN = 4             # ACT-path quarters per tile (450 elems, 1 PSUM bank)
QF = FREE // QN    # 450
NAS = 2            # ACT tiles per group in v4 (pattern char 'A')

_BUILD_CACHE: dict = {}


def _is_gps(t: int) -> bool:
    pat = os.environ.get("K_GPSPAT", "DDG")
    return pat[t % len(pat)] == "G"


def _gps_slots(g: int) -> list:
    return [tq for tq in range(GT) if _is_gps(g * GT + tq)]


def _pat4() -> str:
    return os.environ.get("K_PAT4", "DDGADDGA")


def _kind4(t: int) -> str:
    pat = _pat4()
    return pat[t % len(pat)]


def _act_slots(g: int) -> list:
    return [tq for tq in range(GT) if _kind4(g * GT + tq) == "A"]


def _build(loop_iters: int = 1, expand: bool | None = None,
           in_bufs: int = 4, out_bufs: int = 8):
    """Build + compile the per-core Bacc program (identical on all 8 cores)."""
    if os.environ.get("K_MODE", "v3") == "v4":
        return _build_v4(loop_iters)
    if expand is None:
        expand = bool(int(os.environ.get("K_EXPAND", "1")))
    nb = int(os.environ.get("K_STORE_BATCH", "2"))
    in_bufs = int(os.environ.get("K_IN_BUFS", str(in_bufs)))
    out_bufs = int(os.environ.get("K_OUT_BUFS", str(out_bufs)))
    store_alt = bool(int(os.environ.get("K_STORE_ALT", "1")))
    etop_bufs = int(os.environ.get("K_ETOP_BUFS", "2"))
    ebot_bufs = int(os.environ.get("K_EBOT_BUFS", "2"))
    assert GT % nb == 0
    nc = bacc.Bacc("TRN2", target_bir_lowering=False, debug=False, num_devices=NCORES)
    dt = mybir.dt.float16
    f32 = mybir.dt.float32

    out_d = nc.dram_tensor("out", [BPC, P, FREE], dt, kind="ExternalOutput")
    if expand:
        # raw[b, s, k, c*TCOL + t] = seq_pairs[b, c, t*128 + k, s] (0-padded)
        raw_d = nc.dram_tensor("raw", [BPC, 2, 128, C * TCOL], dt,
                               kind="ExternalInput")
        # band1[k, x] = [k == x]; band2[k, x] = [k == x - 128]
        band1_d = nc.dram_tensor("band1", [128, 128 + WPAD], dt,
                                 kind="ExternalInput")
        band2_d = nc.dram_tensor("band2", [128, 128 + WPAD], dt,
                                 kind="ExternalInput")
        # Host-packed window slices for the GPSIMD-assigned tiles only.
        inwg_d = nc.dram_tensor("inwg", [BPC, NG, 128, NGS * 2 * CW], dt,
                                kind="ExternalInput")
    else:
        inw_d = nc.dram_tensor("inw", [BPC, NG, 128, 2 * GW], dt,
                               kind="ExternalInput")
        ident_d = nc.dram_tensor("ident", [128, 128], dt,
                                 kind="ExternalInput")

    with tile.TileContext(nc) as tc:
        with (
            tc.tile_pool(name="inp", bufs=in_bufs) as inp,
            tc.tile_pool(name="outp", bufs=out_bufs) as outp,
            tc.tile_pool(name="const", bufs=1) as constp,
            tc.psum_pool(name="ps", bufs=2) as psp,
            tc.psum_pool(name="pst", bufs=etop_bufs) as pstp,
            tc.psum_pool(name="psb", bufs=ebot_bufs) as psbp,
        ):
            if expand:
                band1t = constp.tile([128, 128 + WPAD], dt, tag="band1")
                band2t = constp.tile([128, 128 + WPAD], dt, tag="band2")
                nc.scalar.dma_start(band1t[:], band1_d[:, :])
                nc.scalar.dma_start(band2t[:], band2_d[:, :])
            else:
                identt = constp.tile([128, 128], dt, tag="ident")
                nc.scalar.dma_start(identt[:], ident_d[:, :])

            def _bcast(src, axis):
                # [128, CW] window tile (c-major) -> AP [128, C, W, W]
                v = src.rearrange("p (c x) -> p c x", c=C)
                v = v.unsqueeze(3) if axis == 0 else v.unsqueeze(2)
                return v.broadcast_to((128, C, W, W))

            def _expand_windows(rawt, s, g):
                """PE: shift-matmul the raw sequences into window tiles.

                Returns a PSUM tile E [128, GW] f32 with layout
                E[dp, (i*C + c)*GT + tc] = window value i for channel c of
                position (g*GT + tc)*128 + dp.
                """
                pool = pstp if s == 0 else psbp
                e = pool.tile([128, GW], f32, tag=f"e{s}")
                for i in range(W):
                    sl = e[:, i * C * GT : (i + 1) * C * GT]
                    rhs = rawt[:].rearrange("p (c t) -> p c t", c=C)
                    nc.tensor.matmul(
                        sl, band1t[:, i : i + 128],
                        rhs[:, :, g * GT : g * GT + GT],
                        start=True, stop=(i == 0),
                    )
                    if i > 0:  # rows dp >= 128 - i wrap into the next column
                        nc.tensor.matmul(
                            sl, band2t[:, i : i + 128],
                            rhs[:, :, g * GT + 1 : g * GT + GT + 1],
                            start=False, stop=True,
                        )
                return e

            def _body(_it=None):
                for b in range(BPC):
                    if expand:
                        rawts = []
                        for s in range(2):
                            rt = inp.tile([128, C * TCOL], dt, tag=f"raw{s}")
                            nc.scalar.dma_start(rt[:], raw_d[b, s])
                            rawts.append(rt)
                    for g in range(NG):
                        if expand:
                            slots = _gps_slots(g)
                            inwgt = inp.tile([128, NGS * 2 * CW], dt,
                                             tag="inwg")
                            nc.scalar.dma_start(inwgt[:], inwg_d[b, g])
                            etop = _expand_windows(rawts[0], 0, g)
                            ebot = _expand_windows(rawts[1], 1, g)
                            # ACT: reorder bottom windows PSUM->SBUF into
                            # (tc, c, j) layout (dense j for DVE reads).
                            swb = inp.tile([128, GW], dt, tag="swb")
                            src = ebot[:].rearrange(
                                "p (i c t) -> p t c i", c=C, i=W
                            )
                            nc.scalar.copy(
                                swb[:].rearrange("p (t c j) -> p t c j",
                                                 c=C, t=GT), src
                            )
                        else:
                            inwt = inp.tile([128, 2 * GW], dt, tag="inw")
                            nc.scalar.dma_start(inwt[:], inw_d[b, g])
                        pair_order = list(range(0, GT, nb))
                        if bool(int(os.environ.get("K_GFIRST", "0"))):
                            pair_order.sort(key=lambda q0: 0 if any(
                                _is_gps(g * GT + q0 + u) for u in range(nb)
                            ) else 1)
                        for tq0 in pair_order:
                            ot = outp.tile([128, nb * FREE], dt, tag="ot")
                            for u in range(nb):
                                tq = tq0 + u
                                t = g * GT + tq
                                o = ot[:, u * FREE : (u + 1) * FREE].rearrange(
                                    "p (c i j) -> p c i j", c=C, i=W
                                )
                                if expand:
                                    if _is_gps(t):
                                        gi = slots.index(tq)
                                        a_src = inwgt[:, gi * 2 * CW :
                                                      gi * 2 * CW + CW]
                                        b_src = inwgt[:, gi * 2 * CW + CW :
                                                      (gi + 1) * 2 * CW]
                                        nc.gpsimd.tensor_mul(
                                            o, _bcast(a_src, 0),
                                            _bcast(b_src, 1)
                                        )
                                    else:
                                        # A: top windows straight from PSUM
                                        # E[dp, (i c tc)] -> [128, C, W, W]
                                        a = (
                                            etop[:]
                                            .rearrange("p (i c t) -> p c i t",
                                                       c=C, i=W)
                                            [:, :, :, tq : tq + 1]
                                            .broadcast_to((128, C, W, W))
                                        )
                                        bb = (
                                            swb[:, tq * CW : (tq + 1) * CW]
                                            .rearrange("p (c j) -> p c j",
                                                       c=C)
                                            .unsqueeze(2)
                                            .broadcast_to((128, C, W, W))
                                        )
                                        nc.vector.tensor_mul(o, a, bb)
                                elif _is_gps(t):
                                    a_src = inwt[:, tq * CW : (tq + 1) * CW]
                                    b_src = inwt[:, GW + tq * CW :
                                                 GW + (tq + 1) * CW]
                                    nc.gpsimd.tensor_mul(
                                        o, _bcast(a_src, 0), _bcast(b_src, 1)
                                    )
                                else:
                                    a_src = inwt[:, tq * CW : (tq + 1) * CW]
                                    b_src = inwt[:, GW + tq * CW :
                                                 GW + (tq + 1) * CW]
                                    psa = psp.tile([128, CW], f32, tag="psa")
                                    nc.tensor.matmul(
                                        psa[:], identt[:], a_src,
                                        start=True, stop=True,
                                    )
                                    nc.vector.tensor_mul(
                                        o, _bcast(psa[:], 0), _bcast(b_src, 1)
                                    )
                            t0 = g * GT + tq0
                            rows = min(nb * 128, P - t0 * 128)
                            full = rows // 128
                            alt_by = os.environ.get("K_ALT_BY", "pair")
                            ph = (g if alt_by == "g" else tq0 // nb) % 2
                            st_eng = (nc.scalar if store_alt and ph
                                      else nc.sync)
                            sfrac = int(os.environ.get("K_STORE_FRAC", "1"))
                            fr = FREE // sfrac
                            if full:
                                dst = out_d[
                                    b, t0 * 128 : t0 * 128 + full * 128, :fr
                                ].rearrange("(u p) f -> p u f", u=full)
                                if sfrac == 1:
                                    st_eng.dma_start(dst, ot[:, : full * FREE])
                                else:
                                    src_ap = ot[:].rearrange(
                                        "p (u f) -> p u f", u=nb
                                    )[:, :full, :fr]
                                    st_eng.dma_start(dst, src_ap)
                            if rows % 128:  # partial last tile (t == NT-1)
                                st_eng.dma_start(
                                    out_d[b, t0 * 128 + full * 128 :
                                          t0 * 128 + rows, :],
                                    ot[: rows % 128,
                                       full * FREE : (full + 1) * FREE],
                                )

            if loop_iters == 1:
                _body()
            else:
                with tc.For_i(0, loop_iters, 1) as it:
                    _body(it)
    nc.compile()
    return nc


def _build_v4(loop_iters: int = 1):
    """v4: three-engine multiply split (DVE / GPSIMD / ACT-square path).

    ACT tiles use x*y = (sqrt(.5)*(x+y))**2 - .5*x^2 - .5*y^2: PE builds
    S = x + y in PSUM with two accumulating broadcast matmuls, ACT squares
    PSUM->PSUM, PE subtracts host-packed squared-window terms (fp16 hi+lo
    split keeps the identity exact to ~2^-22), ACT copies to the fp16
    output tile. DVE keeps its PSUM-operand trick; GPSIMD unchanged.
    """
    nb = int(os.environ.get("K_STORE_BATCH", "2"))
    in_bufs = int(os.environ.get("K_IN_BUFS", "4"))
    out_bufs = int(os.environ.get("K_OUT_BUFS", "8"))
    store_alt = bool(int(os.environ.get("K_STORE_ALT", "0")))
    resid = bool(int(os.environ.get("K_RESID", "1")))
    sab_bufs = int(os.environ.get("K_SAB_BUFS", "3"))
    pat = _pat4()
    assert GT % nb == 0 and len(pat) % GT == 0
    assert all(len(_act_slots(g)) <= NAS for g in range(NG))
    nc = bacc.Bacc("TRN2", target_bir_lowering=False, debug=False,
                   num_devices=NCORES)
    dt = mybir.dt.float16
    f32 = mybir.dt.float32
    SQRT_HALF = 0.7071067811865476

    out_d = nc.dram_tensor("out", [BPC, P, FREE], dt, kind="ExternalOutput")
    inw_d = nc.dram_tensor("inw", [BPC, NG, 128, 2 * GW], dt,
                           kind="ExternalInput")
    # sqw[b, g, :, slot*4CW ...]: (tsq_hi, tsq_lo, bsq_hi, bsq_lo) windows
    sqw_d = nc.dram_tensor("sqw", [BPC, NG, 128, NAS * 4 * CW], dt,
                           kind="ExternalInput")
    ident_d = nc.dram_tensor("ident", [128, 128], dt, kind="ExternalInput")
    nident_d = nc.dram_tensor("nident", [128, 128], dt, kind="ExternalInput")

    with tile.TileContext(nc) as tc:
        with (
            tc.tile_pool(name="inp", bufs=in_bufs) as inp,
            tc.tile_pool(name="outp", bufs=out_bufs) as outp,
            tc.tile_pool(name="const", bufs=1) as constp,
            tc.psum_pool(name="psa", bufs=2) as psap,
            tc.psum_pool(name="sa", bufs=2) as sap,
            tc.psum_pool(name="sb", bufs=4) as sbp,
        ):
            identt = constp.tile([128, 128], dt, tag="ident")
            nidentt = constp.tile([128, 128], dt, tag="nident")
            nc.scalar.dma_start(identt[:], ident_d[:, :])
            nc.scalar.dma_start(nidentt[:], nident_d[:, :])

            def _bc(src, axis, nch=C):
                v = src.rearrange("p (c x) -> p c x", c=nch)
                v = v.unsqueeze(3) if axis == 0 else v.unsqueeze(2)
                return v.broadcast_to((128, nch, W, W))

            def _act_tile(inwt, sqwt, gi, tq, ot, u):
                # Stage-parallel emission: engines execute their queues in
                # order, so emitting all S-builds, then all squares, then all
                # subtracts, then all copies lets the 4 quarters pipeline
                # across PE<->ACT instead of serializing per quarter.
                a_src = inwt[:, tq * CW : (tq + 1) * CW]
                b_src = inwt[:, GW + tq * CW : GW + (tq + 1) * CW]
                base = gi * 4 * CW
                CPQ = C // QN  # channels per quarter = 2
                sAs, sBs = [], []
                for q in range(QN):
                    c0 = q * CPQ
                    asl = a_src[:, c0 * W : (c0 + CPQ) * W]
                    bsl = b_src[:, c0 * W : (c0 + CPQ) * W]
                    sA = sap.tile([128, QF], f32, tag="sA")
                    nc.tensor.matmul(sA[:], identt[:], _bc(asl, 0, CPQ),
                                     start=True, stop=False)
                    nc.tensor.matmul(sA[:], identt[:], _bc(bsl, 1, CPQ),
                                     start=False, stop=True)
                    sAs.append(sA)
                    sB = sbp.tile([128, QF], f32, tag="sB")
                    nc.scalar.activation(
                        sB[:], sA[:], mybir.ActivationFunctionType.Square,
                        scale=SQRT_HALF,
                    )
                    sBs.append(sB)
                terms = [(0, 0), (2, 1)]  # (sqw section, bcast axis)
                if resid:
                    terms += [(1, 0), (3, 1)]
                for q in range(QN):
                    c0 = q * CPQ
                    for sec, ax in terms:
                        tsl = sqwt[:, base + sec * CW + c0 * W :
                                   base + sec * CW + (c0 + CPQ) * W]
                        nc.tensor.matmul(sBs[q][:], nidentt[:],
                                         _bc(tsl, ax, CPQ),
                                         start=False,
                                         stop=(sec == terms[-1][0]),
                                         skip_group_check=True)
                for q in range(QN):
                    nc.scalar.copy(
                        ot[:, u * FREE + q * QF : u * FREE + (q + 1) * QF],
                        sBs[q][:],
                    )

            def _body(_it=None):
                for b in range(BPC):
                    for g in range(NG):
                        aslots = _act_slots(g)
                        ld_eng = {"scalar": nc.scalar, "sync": nc.sync,
                                  "tensor": nc.tensor}[
                            os.environ.get("K_LOAD_ENG", "sync")]
                        inwt = inp.tile([128, 2 * GW], dt, tag="inw")
                        ld_eng.dma_start(inwt[:], inw_d[b, g])
                        sqwt = inp.tile([128, NAS * 4 * CW], dt, tag="sqw")
                        ld_eng.dma_start(sqwt[:], sqw_d[b, g])
                        dslots = [tq for tq in range(GT)
                                  if _kind4(g * GT + tq) == "D"]
                        assert len(dslots) * CW <= 512
                        psg = psap.tile([128, len(dslots) * CW], f32,
                                        tag="psg")
                        for di, tq in enumerate(dslots):
                            nc.tensor.matmul(
                                psg[:, di * CW : (di + 1) * CW], identt[:],
                                inwt[:, tq * CW : (tq + 1) * CW],
                                start=True, stop=True,
                            )
                        for tq0 in range(0, GT, nb):
                            ot = outp.tile([128, nb * FREE], dt, tag="ot")
                            for u in range(nb):
                                tq = tq0 + u
                                a_src = inwt[:, tq * CW : (tq + 1) * CW]
                                b_src = inwt[:, GW + tq * CW :
                                             GW + (tq + 1) * CW]
                                o = ot[:, u * FREE : (u + 1) * FREE].rearrange(
                                    "p (c i j) -> p c i j", c=C, i=W
                                )
                                kind = _kind4(g * GT + tq)
                                if kind == "G":
                                    nc.gpsimd.tensor_mul(
                                        o, _bc(a_src, 0), _bc(b_src, 1)
                                    )
                                elif kind == "A":
                                    _act_tile(inwt, sqwt, aslots.index(tq),
                                              tq, ot, u)
                                else:
                                    di = dslots.index(tq)
                                    pa = psg[:, di * CW : (di + 1) * CW]
                                    nc.vector.tensor_mul(
                                        o, _bc(pa, 0), _bc(b_src, 1)
                                    )
                            t0 = g * GT + tq0
                            rows = min(nb * 128, P - t0 * 128)
                            full = rows // 128
                            st_eng = (nc.scalar if store_alt and
                                      (tq0 // nb) % 2 else nc.sync)
                            if full:
                                dst = out_d[
                                    b, t0 * 128 : t0 * 128 + full * 128, :
                                ].rearrange("(u p) f -> p u f", u=full)
                                st_eng.dma_start(dst, ot[:, : full * FREE])
                            if rows % 128:
                                st_eng.dma_start(
                                    out_d[b, t0 * 128 + full * 128 :
                                          t0 * 128 + rows, :],
                                    ot[: rows % 128,
                                       full * FREE : (full + 1) * FREE],
                                )

            if loop_iters == 1:
                _body()
            else:
                with tc.For_i(0, loop_iters, 1) as it:
                    _body(it)
    nc.compile()
    return nc


def _get_built(loop_iters: int = 1):
    nc = _BUILD_CACHE.get(loop_iters)
    if nc is None:
        nc = _build(loop_iters)
        _BUILD_CACHE[loop_iters] = nc
    return nc


def _windows(seq_pairs: np.ndarray) -> np.ndarray:
    """[b, g, p, s, tq, c, i] fp32 sliding windows (0-padded past L)."""
    sp = np.ascontiguousarray(seq_pairs, dtype=np.float32)
    padded = np.zeros((B, C, L + WPAD, 2), np.float32)
    padded[:, :, :L] = sp
    win = sliding_window_view(padded, W, axis=2)  # [B, C, L, 2, W]
    v = win.reshape(B, C, NG, GT, 128, 2, W)
    return v.transpose(0, 2, 4, 5, 3, 1, 6)


def _inputs_for(seq_pairs: np.ndarray, expand: bool | None = None) -> dict:
    """Full-batch device input arrays for the compiled program."""
    if os.environ.get("K_MODE", "v3") == "v4":
        v = _windows(seq_pairs)  # [b,g,p,s,tq,c,i] f32
        v16 = v.astype(np.float16)
        inw = np.ascontiguousarray(v16.reshape(B, NG, 128, 2 * GW))
        # squared windows of the fp16 values, split hi+lo so the device's
        # 0.5*S^2 - 0.5*x^2 - 0.5*y^2 identity is exact to ~2^-22
        sq = np.square(v16.astype(np.float32))  # [b,g,p,s,tq,c,i]
        hi = sq.astype(np.float16)
        lo = (sq - hi.astype(np.float32)).astype(np.float16)
        sqw = np.zeros((B, NG, 128, NAS * 4 * CW), np.float16)
        for g in range(NG):
            for gi, tq in enumerate(_act_slots(g)):
                base = gi * 4 * CW
                for sec, (s, arr) in enumerate(
                    [(0, hi), (0, lo), (1, hi), (1, lo)]
                ):
                    sqw[:, g, :, base + sec * CW : base + (sec + 1) * CW] = (
                        arr[:, g, :, s, tq].reshape(B, 128, CW))
        ident = np.eye(128, dtype=np.float16)
        return {"inw": inw, "sqw": sqw, "ident": ident,
                "nident": (-0.5 * ident).astype(np.float16)}
    if expand is None:
        expand = bool(int(os.environ.get("K_EXPAND", "1")))
    if not expand:
        v = _windows(seq_pairs)  # [b,g,p,s,tq,c,i]
        inw = np.ascontiguousarray(v.reshape(B, NG, 128, 2 * GW)
                                   .astype(np.float16))
        return {"inw": inw, "ident": np.eye(128, dtype=np.float16)}
    sp = np.ascontiguousarray(seq_pairs, dtype=np.float32)
    # raw[b, s, k, c*TCOL + t] = seq[b, c, t*128 + k, s], zero-padded
    padded = np.zeros((B, C, TCOL * 128, 2), np.float32)
    padded[:, :, :L] = sp
    raw = padded.reshape(B, C, TCOL, 128, 2).transpose(0, 4, 3, 1, 2)
    raw = np.ascontiguousarray(raw.reshape(B, 2, 128, C * TCOL)
                               .astype(np.float16))
    band1 = np.zeros((128, 128 + WPAD), np.float16)
    band1[:, :128] = np.eye(128, dtype=np.float16)
    band2 = np.zeros((128, 128 + WPAD), np.float16)
    for k in range(WPAD):
        band2[k, 128 + k] = 1.0
    v = _windows(seq_pairs)  # [b,g,p,s,tq,c,i]
    inwg = np.zeros((B, NG, 128, NGS * 2 * CW), np.float16)
    for g in range(NG):
        for gi, tq in enumerate(_gps_slots(g)):
            w = v[:, g, :, :, tq]  # [B, 128, s, c, i]
            inwg[:, g, :, gi * 2 * CW : gi * 2 * CW + CW] = (
                w[:, :, 0].reshape(B, 128, CW))
            inwg[:, g, :, gi * 2 * CW + CW : (gi + 1) * 2 * CW] = (
                w[:, :, 1].reshape(B, 128, CW))
    return {"raw": raw, "band1": band1, "band2": band2, "inwg": inwg}


def _percore(inputs: dict, k: int) -> dict:
    out = {}
    for name, arr in inputs.items():
        if name in ("ident", "nident", "band1", "band2"):
            out[name] = arr
        else:
            out[name] = arr[k * BPC : (k + 1) * BPC]
    return out


def kernel(seq_pairs: np.ndarray) -> np.ndarray:
    assert tuple(np.shape(seq_pairs)) == (B, C, L, 2), (
        f"expected seq_pairs shape {(B, C, L, 2)}, got {np.shape(seq_pairs)}"
    )
    inputs = _inputs_for(seq_pairs)
    nc = _get_built()
    in_maps = [_percore(inputs, k) for k in range(NCORES)]
    last_err = None
    for _attempt in range(3):
        try:
            res = run_bass_kernel_spmd(nc, in_maps, list(range(NCORES))).results
            break
        except Exception as err:  # transient axon/PJRT hiccups — retry
            last_err = err
    else:
        raise last_err
    out = np.concatenate([res[k]["out"] for k in range(NCORES)], axis=0)
    return np.ascontiguousarray(out.astype(np.float32))
